# revision 18
# baseline (speedup 1.0000x reference)
"""Cosformer attention Bass kernel for 8 trn2 NeuronCores.

Sharding: core c handles batch c//2, sequence half c%2 (1024 tokens).
Per-head linear-attention state (kv, ksum) is AllReduce'd (bf16) between
the two cores sharing a batch.

v4 design:
- LN1 folded into the q GEMM: q runs W-stationary on the feature-major x
  plus a rank-1 cq (x) (-mu) correction matmul into the same PSUM group;
  rstd is folded into the sin/cos multiplier tiles (rstd, s, c > 0 commute
  with ReLU). No qn transposes.
- PE order: v GEMM, k GEMM with per-head kv matmuls interleaved per
  tile-pair (AllReduce triggers right after the k GEMM), q GEMM (covers
  the AllReduce), then per token tile: attn matmuls, PE-transpose of xh
  (identity matmul), output GEMM.
- Consecutive matmuls always alternate PSUM banks (pair loops) so the PE
  pipelines; kv/attn matmul orders are bank-interleaved.
- No gpsimd elementwise ops (Pool engine is ~10-25x slower than DVE).
  gpsimd does DMA issue, memset, partition_broadcast, collectives only.
- Scalar engine does all func(in*scale+bias) epilogues: v/k epilogues,
  qn and xh (LayerNorm apply via Identity with per-token scale/bias),
  relu(q), PSUM->SBUF copies. Vector does stats, sin/cos mults, z, y.
"""

import sys

for _p in ('/opt/trn_rl_repo',):
    if _p not in sys.path:
        sys.path.insert(0, _p)

import os

os.environ.setdefault('NEURON_RT_RESET_CORES', '1')

# The image's antenv may lack axon_hooks (needed for trace=True); register
# a stub module so `from antenv.axon_hooks import ...` works.
if 'antenv.axon_hooks' not in sys.modules:
    try:
        import antenv.axon_hooks  # noqa: F401
    except ImportError:
        import types as _types

        _mod = _types.ModuleType('antenv.axon_hooks')
        _mod._hook = None

        def _set_hook(h):
            _mod._hook = h

        def _get_hook():
            return _mod._hook

        _mod.set_axon_ntff_profile_hook = _set_hook
        _mod.get_axon_ntff_profile_hook = _get_hook
        sys.modules['antenv.axon_hooks'] = _mod

import numpy as np
import ml_dtypes

import concourse.bass as bass  # noqa: F401
import concourse.tile as tile
from concourse import bacc, mybir
from concourse.alu_op_type import AluOpType
from concourse.bass_utils import run_bass_kernel_spmd

BF16 = ml_dtypes.bfloat16
FP32 = mybir.dt.float32
BF = mybir.dt.bfloat16
AF = mybir.ActivationFunctionType

L, N, E, H, D = 2048, 4, 1024, 16, 64
T = 1024            # tokens per core
NT = T // 128       # 8 token tiles
NK = E // 128       # 8 contraction tiles
NJ = E // 128       # 8 output-feature tiles
NCORES = 8
EPS_LN = 1e-5
EPS_ATTN = 1e-6

# kv psum: 3 banks x (6|6|4) heads; emission order interleaves banks
KVSLOT = [(h // 6, (h % 6) * 65) for h in range(H)]
KV_ORDER = [0, 6, 12, 1, 7, 13, 2, 8, 14, 3, 9, 15, 4, 10, 5, 11]

_BUILD_CACHE = {}


def _build_program(flags):
    """Build the SPMD Bass program. flags: (has_g1b1, has_qb, has_kb, has_vb, has_b2o)."""
    has_g1b1, has_qb, has_kb, has_vb, has_b2o = flags

    nc = bacc.Bacc("TRN2", target_bir_lowering=False, debug=False,
                   num_devices=NCORES)

    # ---- DRAM I/O ----
    d_x_tm = nc.dram_tensor('x_tmb', [T, E], BF, kind='ExternalInput')
    d_x_fm = nc.dram_tensor('x_fm', [E, T], BF, kind='ExternalInput')
    d_wq = nc.dram_tensor('wq', [E, E], BF, kind='ExternalInput')
    d_wk = nc.dram_tensor('wk', [E, E], BF, kind='ExternalInput')
    d_wv = nc.dram_tensor('wv', [E, E], BF, kind='ExternalInput')
    d_wo = nc.dram_tensor('wo2', [E, E], BF, kind='ExternalInput')
    d_sb = nc.dram_tensor('s_bcast', [128, T], BF, kind='ExternalInput')
    d_cb = nc.dram_tensor('c_bcast', [128, T], BF, kind='ExternalInput')
    d_scol = nc.dram_tensor('s_cols', [128, NT], FP32, kind='ExternalInput')
    d_ccol = nc.dram_tensor('c_cols', [128, NT], FP32, kind='ExternalInput')
    d_cq = nc.dram_tensor('cq_row', [1, E], BF, kind='ExternalInput')
    d_ident = nc.dram_tensor('ident', [128, 128], BF, kind='ExternalInput')
    d_g1b = nc.dram_tensor('g1_b', [128, E], FP32, kind='ExternalInput') if has_g1b1 else None
    d_b1b = nc.dram_tensor('b1_b', [128, E], FP32, kind='ExternalInput') if has_g1b1 else None
    d_qbc = nc.dram_tensor('qb_cols', [128, NJ], FP32, kind='ExternalInput') if has_qb else None
    d_kbb = nc.dram_tensor('kb_b', [128, E], FP32, kind='ExternalInput') if has_kb else None
    d_vbb = nc.dram_tensor('vb_b', [128, E], FP32, kind='ExternalInput') if has_vb else None
    d_b2ob = nc.dram_tensor('b2o_b', [128, E], FP32, kind='ExternalInput') if has_b2o else None
    d_out = nc.dram_tensor('out', [T, E], FP32, kind='ExternalOutput')

    RG = [[0, 1], [2, 3], [4, 5], [6, 7]]

    with tile.TileContext(nc) as tc:
        with (
            tc.tile_pool(name='persist', bufs=1) as pp,
            tc.tile_pool(name='wpool', bufs=2) as wp,
            tc.tile_pool(name='dram', bufs=1, space='DRAM') as dp,
        ):
            # ---- priority DMAs: first GEMM inputs ----
            xfm = pp.tile([128, NK, T], BF, tag='xfm')
            xfm_src = d_x_fm[:].rearrange('(k p) t -> p k t', p=128)
            for k in range(NK):
                nc.sync.dma_start(out=xfm[:, k, :], in_=xfm_src[:, k, :])
            wv_t = wp.tile([128, NK, E], BF, tag='W', name='wv')
            wv_src = d_wv[:].rearrange('(k p) e -> p k e', p=128)
            for k in range(NK):
                eng = nc.gpsimd if k % 2 == 0 else nc.scalar
                eng.dma_start(out=wv_t[:, k, :], in_=wv_src[:, k, :])
            # x token-major (bf16) for LN1 stats + qn residual; scalar queue
            xt_sb = pp.tile([128, NT, E], BF, tag='xt')
            for i in range(NT):
                nc.scalar.dma_start(out=xt_sb[:, i, :],
                                    in_=d_x_tm[i * 128:(i + 1) * 128, :])

            # ---- constants (sync queue, after xfm) ----
            scol = pp.tile([128, NT], FP32, tag='scol')
            ccol = pp.tile([128, NT], FP32, tag='ccol')
            sbt = pp.tile([128, T], BF, tag='sbt')
            cbt = pp.tile([128, T], BF, tag='cbt')
            cq_sb = pp.tile([1, E], BF, tag='cq')
            ident = pp.tile([128, 128], BF, tag='ident')
            eps1 = pp.tile([128, 1], FP32, tag='eps1')
            nc.sync.dma_start(out=scol, in_=d_scol[:])
            nc.sync.dma_start(out=ccol, in_=d_ccol[:])
            nc.sync.dma_start(out=sbt, in_=d_sb[:])
            nc.sync.dma_start(out=cbt, in_=d_cb[:])
            nc.sync.dma_start(out=cq_sb, in_=d_cq[:])
            nc.sync.dma_start(out=ident, in_=d_ident[:])
            nc.vector.memset(eps1, EPS_LN)

            # wk on sync after consts (needed when the k GEMM starts)
            wk_t = wp.tile([128, NK, E], BF, tag='W', name='wk')
            wk_src = d_wk[:].rearrange('(k p) e -> p k e', p=128)
            for k in range(NK):
                nc.sync.dma_start(out=wk_t[:, k, :], in_=wk_src[:, k, :])

            g1b = b1b = qbc = kbb = vbb = b2ob = None
            if has_g1b1:
                g1b = pp.tile([128, E], FP32, tag='g1b')
                b1b = pp.tile([128, E], FP32, tag='b1b')
                nc.gpsimd.dma_start(out=g1b, in_=d_g1b[:])
                nc.gpsimd.dma_start(out=b1b, in_=d_b1b[:])
            if has_qb:
                qbc = pp.tile([128, NJ], FP32, tag='qbc')
                nc.gpsimd.dma_start(out=qbc, in_=d_qbc[:])
            if has_kb:
                kbb = pp.tile([128, E], FP32, tag='kbb')
                nc.gpsimd.dma_start(out=kbb, in_=d_kbb[:])
            if has_vb:
                vbb = pp.tile([128, E], FP32, tag='vbb')
                nc.gpsimd.dma_start(out=vbb, in_=d_vbb[:])
            if has_b2o:
                b2ob = pp.tile([128, E], FP32, tag='b2ob')
                nc.gpsimd.dma_start(out=b2ob, in_=d_b2ob[:])

            # ---- persistent activation tiles ----
            qn_sb = pp.tile([128, NT, E], BF, tag='qn')        # LN1(x) residual
            qq = pp.tile([128, H, T], BF, tag='qq')            # q_ per head, fm
            kvb = pp.tile([128, H * 65], BF, tag='kvb')        # reduced kv
            kvp = pp.tile([128, H * 65], BF, tag='kvp')        # local partial
            xhT = pp.tile([128, NK, T], BF, tag='xhT')         # xh feature-major
            nmu_c = pp.tile([128, 128], BF, tag='nmu_c')       # [-mu|pad|rstd|pad]
            nmu_row = pp.tile([1, T], BF, tag='nmu_row')       # -mu token-major
            rstd_row = pp.tile([1, T], BF, tag='rstd_row')
            rstd_b = pp.tile([128, T], BF, tag='rstd_b')       # rstd part-bcast
            srt = pp.tile([128, T], BF, tag='srt')             # s*rstd (or s)
            crt = pp.tile([128, T], BF, tag='crt')             # c*rstd (or c)
            mvs = pp.tile([128, NT, 2], FP32, tag='mvs')       # LN1 (mu, var)
            rstds = pp.tile([128, NT], FP32, tag='rstds')      # LN1 rstd cols
            nmrs = pp.tile([128, NT], FP32, tag='nmrs')        # -mu*rstd cols

            # ---- DRAM scratch ----
            nm_dram = dp.tile([2 * NT, 128], BF)     # transposed [-mu|rstd]
            kv_cc_in = dp.tile([128, H * 65], BF)
            kv_cc_out = dp.tile([128, H * 65], BF)

            nc.gpsimd.memset(nmu_c, 0.0)

            # ============ Phase A1: LN1 stats (vector only; overlaps B1) ===
            with tc.tile_pool(name='ln1', bufs=2) as ap:
                for i in range(NT):
                    st = ap.tile([128, 2, 6], FP32, tag='st')
                    xg = xt_sb[:, i, :].rearrange('p (g d) -> p g d', g=2)
                    nc.vector.bn_stats(out=st[:, 0, :], in_=xg[:, 0, :])
                    nc.vector.bn_stats(out=st[:, 1, :], in_=xg[:, 1, :])
                    nc.vector.bn_aggr(out=mvs[:, i, :], in_=st)
                    nc.vector.tensor_scalar(out=nmu_c[:, i:i + 1],
                                            in0=mvs[:, i, 0:1],
                                            scalar1=-1.0, scalar2=None,
                                            op0=AluOpType.mult)

                # ======== Phase B1: v GEMM (per-tile k-inner ch-pairs) =====
                with tc.tile_pool(name='bphase', bufs=1) as bp:
                    v_aug = bp.tile([128, NT, H, 65], BF, tag='vaug')
                    nc.gpsimd.memset(v_aug[:, :, :, 64:65], 1.0)

                    def gemm_pair(psb, w_t, i, nametag):
                        p0 = psb.tile([128, 512], FP32, tag='psB',
                                      name=f'{nametag}_{i}_0')
                        p1 = psb.tile([128, 512], FP32, tag='psB',
                                      name=f'{nametag}_{i}_1')
                        for k in range(NK):
                            lh = xfm[:, k, i * 128:(i + 1) * 128]
                            nc.tensor.matmul(p0, lhsT=lh, rhs=w_t[:, k, 0:512],
                                             start=(k == 0), stop=(k == NK - 1))
                            nc.tensor.matmul(p1, lhsT=lh, rhs=w_t[:, k, 512:1024],
                                             start=(k == 0), stop=(k == NK - 1))
                        return p0, p1

                    def v_epilogue(i, ch, pv):
                        if has_vb:
                            csl = slice(ch * 512, (ch + 1) * 512)
                            nc.vector.tensor_tensor(out=pv, in0=pv, in1=vbb[:, csl],
                                                    op=AluOpType.add)
                        nc.scalar.activation(
                            out=v_aug[:, i, ch * 8:(ch + 1) * 8, 0:64],
                            in_=pv[:].rearrange('p (h d) -> p h d', d=64),
                            func=AF.Copy)

                    with tc.tile_pool(name='psB1', bufs=7, space='PSUM') as psb1:
                        for i in range(NT):
                            p0, p1 = gemm_pair(psb1, wv_t, i, 'pv')
                            v_epilogue(i, 0, p0)
                            v_epilogue(i, 1, p1)

                    # ==== Phase A2: rstd (scalar sqrt after v-epilogues) ===
                    for i in range(NT):
                        nc.scalar.activation(out=rstds[:, i:i + 1],
                                             in_=mvs[:, i, 1:2], func=AF.Sqrt,
                                             bias=eps1, scale=1.0)
                    for i in range(NT):
                        nc.vector.reciprocal(out=rstds[:, i:i + 1],
                                             in_=rstds[:, i:i + 1])
                        nc.vector.tensor_copy(out=nmu_c[:, 64 + i:64 + i + 1],
                                              in_=rstds[:, i:i + 1])
                        nc.vector.tensor_scalar(out=nmrs[:, i:i + 1],
                                                in0=mvs[:, i, 0:1],
                                                scalar1=rstds[:, i:i + 1],
                                                scalar2=-1.0, op0=AluOpType.mult,
                                                op1=AluOpType.mult)

                    # [-mu | rstd] cols -> token-major [1, T] rows (via DRAM)
                    nm128 = ap.tile([128, 128], BF, tag='nm128')
                    nc.sync.dma_start(out=nm128, in_=nmu_c, transpose=True)
                    nc.sync.dma_start(out=nm_dram[0:NT, :], in_=nm128[0:NT, :])
                    nc.sync.dma_start(out=nm_dram[NT:2 * NT, :],
                                      in_=nm128[64:64 + NT, :])
                    nc.sync.dma_start(out=nmu_row,
                                      in_=nm_dram[0:NT, :].rearrange('a b -> (a b)'))
                    nc.sync.dma_start(out=rstd_row,
                                      in_=nm_dram[NT:2 * NT, :].rearrange('a b -> (a b)'))
                    nc.gpsimd.partition_broadcast(rstd_b, rstd_row)

                    # ==== Phase B2: k GEMM + kv accumulation interleaved ===
                    ksc = bp.tile([128, NT, H, 128], BF, tag='ksc')
                    psb2_ctx = (
                        tc.tile_pool(name='psB2', bufs=5, space='PSUM'),
                        tc.tile_pool(name='psC', bufs=1, space='PSUM'),
                    )
                    psb2 = psb2_ctx[0].__enter__()
                    psc = psb2_ctx[1].__enter__()
                    kvps = [psc.tile([128, 512], FP32, tag=f'kv{b}', name=f'kv{b}')
                            for b in range(3)]

                    def k_epilogue(i, ch, pk):
                        if has_kb:
                            csl = slice(ch * 512, (ch + 1) * 512)
                            nc.vector.tensor_tensor(out=pk, in0=pk, in1=kbb[:, csl],
                                                    op=AluOpType.add)
                        pkv = pk[:].rearrange('p (h d) -> p h d', d=64)
                        # relu(k)*s on scalar (s, c > 0 commute with relu)
                        nc.scalar.activation(
                            out=ksc[:, i, ch * 8:(ch + 1) * 8, 0:64], in_=pkv,
                            func=AF.Relu, scale=scol[:, i:i + 1])
                        # relu(k)*c on vector
                        nc.vector.tensor_scalar(
                            out=ksc[:, i, ch * 8:(ch + 1) * 8, 64:128], in0=pkv,
                            scalar1=0.0, scalar2=ccol[:, i:i + 1],
                            op0=AluOpType.max, op1=AluOpType.mult)

                    for i in range(NT):
                        p0, p1 = gemm_pair(psb2, wk_t, i, 'pk')
                        # the kv chain gates the AllReduce: high priority so
                        # the scheduler doesn't defer it behind the q GEMM
                        with tc.high_priority():
                            k_epilogue(i, 0, p0)
                            k_epilogue(i, 1, p1)
                            for h in KV_ORDER:
                                b, off = KVSLOT[h]
                                nc.tensor.matmul(
                                    kvps[b][:, off:off + 65],
                                    lhsT=ksc[:, i, h, :],
                                    rhs=v_aug[:, i, h, :],
                                    start=(i == 0), stop=(i == NT - 1),
                                    skip_group_check=True)

                    # kv psum -> bf16 sbuf -> DRAM -> AllReduce
                    tc_hp = tc.high_priority()
                    tc_hp.__enter__()
                    for b in range(3):
                        nh = 6 if b < 2 else 4
                        nc.scalar.activation(out=kvp[:, b * 390:b * 390 + nh * 65],
                                             in_=kvps[b][:, 0:nh * 65], func=AF.Copy)
                    nc.gpsimd.dma_start(out=kv_cc_in[:], in_=kvp)
                    nc.gpsimd.collective_compute(
                        'AllReduce', AluOpType.add,
                        ins=[kv_cc_in.opt()], outs=[kv_cc_out.opt()],
                        replica_groups=RG)
                    # kvb load on sync: gpsimd must not block on the collective
                    nc.sync.dma_start(out=kvb, in_=kv_cc_out[:])
                    tc_hp.__exit__(None, None, None)
                    psb2_ctx[1].__exit__(None, None, None)
                    psb2_ctx[0].__exit__(None, None, None)

            # wq on gpsimd (reuses wv slot), wo on scalar (reuses wk slot)
            wq_t = wp.tile([128, NK, E], BF, tag='W', name='wq')
            wq_src = d_wq[:].rearrange('(k p) e -> p k e', p=128)
            for k in range(NK):
                nc.gpsimd.dma_start(out=wq_t[:, k, :], in_=wq_src[:, k, :])
            wo_t = wp.tile([128, NK, E], BF, tag='W', name='wo')
            wo_src = d_wo[:].rearrange('(k p) e -> p k e', p=128)
            for k in range(NK):
                nc.scalar.dma_start(out=wo_t[:, k, :], in_=wo_src[:, k, :])

            # folded sin/cos multipliers
            if has_qb:
                nc.vector.tensor_copy(out=srt, in_=sbt)
                nc.vector.tensor_copy(out=crt, in_=cbt)
            else:
                nc.vector.tensor_tensor(out=srt, in0=sbt, in1=rstd_b,
                                        op=AluOpType.mult)
                nc.vector.tensor_tensor(out=crt, in0=cbt, in1=rstd_b,
                                        op=AluOpType.mult)

            # ============ Phase Bq: q GEMM (W-stationary on x_fm) =========
            with (
                tc.tile_pool(name='psQ', bufs=5, space='PSUM') as psq,
                tc.tile_pool(name='qsb', bufs=4) as qsp,
            ):
                for j in range(NJ):
                    pq0 = psq.tile([128, 512], FP32, tag='psQ', name=f'pq_{j}_0')
                    pq1 = psq.tile([128, 512], FP32, tag='psQ', name=f'pq_{j}_1')
                    for k in range(NK):
                        nc.tensor.matmul(pq0,
                                         lhsT=wq_t[:, k, j * 128:(j + 1) * 128],
                                         rhs=xfm[:, k, 0:512],
                                         start=(k == 0), stop=False)
                        nc.tensor.matmul(pq1,
                                         lhsT=wq_t[:, k, j * 128:(j + 1) * 128],
                                         rhs=xfm[:, k, 512:1024],
                                         start=(k == 0), stop=False)
                    # rank-1 LN1-mean correction: pq += cq[j-chunk] (x) (-mu)
                    nc.tensor.matmul(pq0, lhsT=cq_sb[0:1, j * 128:(j + 1) * 128],
                                     rhs=nmu_row[0:1, 0:512],
                                     start=False, stop=True)
                    nc.tensor.matmul(pq1, lhsT=cq_sb[0:1, j * 128:(j + 1) * 128],
                                     rhs=nmu_row[0:1, 512:1024],
                                     start=False, stop=True)
                    for ch, pq in ((0, pq0), (1, pq1)):
                        csl = slice(ch * 512, (ch + 1) * 512)
                        qrel = qsp.tile([128, 512], BF, tag='qrel')
                        if has_qb:
                            nc.vector.tensor_tensor(out=pq, in0=pq,
                                                    in1=rstd_b[:, csl],
                                                    op=AluOpType.mult)
                            nc.scalar.activation(out=qrel, in_=pq, func=AF.Relu,
                                                 bias=qbc[:, j:j + 1])
                        else:
                            nc.scalar.activation(out=qrel, in_=pq, func=AF.Relu)
                        nc.vector.tensor_tensor(
                            out=qq[0:64, 2 * j, csl], in0=qrel[0:64, :],
                            in1=srt[0:64, csl], op=AluOpType.mult)
                        nc.vector.tensor_tensor(
                            out=qq[64:128, 2 * j, csl], in0=qrel[0:64, :],
                            in1=crt[0:64, csl], op=AluOpType.mult)
                        nc.vector.tensor_tensor(
                            out=qq[0:64, 2 * j + 1, csl], in0=qrel[64:128, :],
                            in1=srt[64:128, csl], op=AluOpType.mult)
                        nc.vector.tensor_tensor(
                            out=qq[64:128, 2 * j + 1, csl], in0=qrel[64:128, :],
                            in1=crt[64:128, csl], op=AluOpType.mult)

            # qn residual (scalar Identity: x*rstd + (-mu*rstd)); after the
            # B phase so the scalar queue never blocks B1/B2 epilogues
            if has_g1b1:
                with tc.tile_pool(name='qnp', bufs=2) as qnp:
                    for i in range(NT):
                        tmp = qnp.tile([128, E], FP32, tag='qtmp')
                        nc.vector.tensor_scalar(out=tmp, in0=xt_sb[:, i, :],
                                                scalar1=mvs[:, i, 0:1],
                                                scalar2=rstds[:, i:i + 1],
                                                op0=AluOpType.subtract,
                                                op1=AluOpType.mult)
                        nc.vector.tensor_mul(tmp, tmp, g1b)
                        nc.vector.tensor_tensor(out=qn_sb[:, i, :], in0=tmp,
                                                in1=b1b, op=AluOpType.add)
            else:
                for i in range(NT):
                    nc.scalar.activation(out=qn_sb[:, i, :], in_=xt_sb[:, i, :],
                                         func=AF.Identity,
                                         scale=rstds[:, i:i + 1],
                                         bias=nmrs[:, i:i + 1])

            # ============ Phases E (attn+LN2) / T (PE transpose) / G (out) =
            with (
                tc.tile_pool(name='ef', bufs=3) as efp,
                tc.tile_pool(name='psE', bufs=4, space='PSUM') as pse,
                tc.tile_pool(name='go', bufs=4) as gop,
                tc.tile_pool(name='psG', bufs=2, space='PSUM') as psg,
                tc.tile_pool(name='psT', bufs=2, space='PSUM') as pst,
            ):
                xh_tiles = {}

                def emit_attn_ln2(i):
                    rsl = slice(i * 128, (i + 1) * 128)
                    yt = efp.tile([128, E], BF, tag='yt')
                    dcol = efp.tile([128, H], FP32, tag='dcol')
                    z16 = efp.tile([128, H], FP32, tag='z16')
                    pas = [pse.tile([128, 512], FP32, tag='psE', name=f'pa_{i}_{g}')
                           for g in range(4)]
                    # bank-interleaved emission: head hh of each group first
                    for hh in range(4):
                        for g in range(4):
                            h = 4 * g + hh
                            nc.tensor.matmul(pas[g][:, hh * 65:(hh + 1) * 65],
                                             lhsT=qq[:, h, rsl],
                                             rhs=kvb[:, h * 65:(h + 1) * 65],
                                             start=True, stop=True)
                    for g in range(4):
                        pav = pas[g][:, 0:260].rearrange('p (h c) -> p h c', c=65)
                        nc.scalar.activation(out=dcol[:, g * 4:(g + 1) * 4],
                                             in_=pav[:, :, 64], func=AF.Copy)
                    nc.vector.tensor_scalar(out=z16, in0=dcol, scalar1=EPS_ATTN,
                                            scalar2=None, op0=AluOpType.max)
                    nc.vector.reciprocal(out=z16, in_=z16)
                    ytv = yt[:].rearrange('p (h d) -> p h d', d=64)
                    for g in range(4):
                        pav = pas[g][:, 0:260].rearrange('p (h c) -> p h c', c=65)
                        zb = z16[:, g * 4:(g + 1) * 4].broadcast_to((128, 4, 64))
                        nc.vector.tensor_tensor(out=ytv[:, g * 4:(g + 1) * 4, :],
                                                in0=pav[:, :, 0:64], in1=zb,
                                                op=AluOpType.mult)
                    nc.vector.tensor_tensor(out=yt, in0=yt, in1=qn_sb[:, i, :],
                                            op=AluOpType.add)
                    # LN2
                    st2 = efp.tile([128, 2, 6], FP32, tag='st2')
                    yg = yt[:].rearrange('p (g d) -> p g d', g=2)
                    nc.vector.bn_stats(out=st2[:, 0, :], in_=yg[:, 0, :])
                    nc.vector.bn_stats(out=st2[:, 1, :], in_=yg[:, 1, :])
                    mv2 = efp.tile([128, 2], FP32, tag='mv2')
                    nc.vector.bn_aggr(out=mv2, in_=st2)
                    rstd2 = efp.tile([128, 1], FP32, tag='rstd2')
                    nc.scalar.activation(out=rstd2, in_=mv2[:, 1:2], func=AF.Sqrt,
                                         bias=eps1, scale=1.0)
                    nc.vector.reciprocal(out=rstd2, in_=rstd2)
                    nmr2 = efp.tile([128, 1], FP32, tag='nmr2')
                    nc.vector.tensor_scalar(out=nmr2, in0=mv2[:, 0:1],
                                            scalar1=rstd2, scalar2=-1.0,
                                            op0=AluOpType.mult, op1=AluOpType.mult)
                    xh = efp.tile([128, E], BF, tag='xh')
                    nc.scalar.activation(out=xh, in_=yt, func=AF.Identity,
                                         scale=rstd2, bias=nmr2)
                    xh_tiles[i] = xh

                def emit_T(i):
                    # PE transpose of xh tile i into xhT (feature-major)
                    xh = xh_tiles.pop(i)
                    for j in range(NJ):
                        pt = pst.tile([128, 128], BF, tag='psT',
                                      name=f'pt_{i}_{j}')
                        nc.tensor.transpose(pt, xh[:, j * 128:(j + 1) * 128],
                                            ident)
                        nc.vector.tensor_copy(
                            out=xhT[:, j, i * 128:(i + 1) * 128], in_=pt)

                def emit_o(i):
                    po0 = psg.tile([128, 512], FP32, tag='psG', name=f'po_{i}_0')
                    po1 = psg.tile([128, 512], FP32, tag='psG', name=f'po_{i}_1')
                    for k in range(NK):
                        nc.tensor.matmul(po0,
                                         lhsT=xhT[:, k, i * 128:(i + 1) * 128],
                                         rhs=wo_t[:, k, 0:512],
                                         start=(k == 0), stop=(k == NK - 1))
                        nc.tensor.matmul(po1,
                                         lhsT=xhT[:, k, i * 128:(i + 1) * 128],
                                         rhs=wo_t[:, k, 512:1024],
                                         start=(k == 0), stop=(k == NK - 1))
                    for ch, po in ((0, po0), (1, po1)):
                        csl = slice(ch * 512, (ch + 1) * 512)
                        ot = gop.tile([128, 512], FP32, tag='ot')
                        if has_b2o:
                            nc.vector.tensor_tensor(out=ot, in0=po,
                                                    in1=b2ob[:, csl],
                                                    op=AluOpType.add)
                        else:
                            nc.scalar.activation(out=ot, in_=po, func=AF.Copy)
                        oeng = nc.sync if ch == 0 else nc.gpsimd
                        oeng.dma_start(out=d_out[i * 128:(i + 1) * 128, csl],
                                       in_=ot)

                emit_attn_ln2(0)
                emit_T(0)
                for i in range(1, NT):
                    emit_attn_ln2(i)
                    emit_o(i - 1)
                    emit_T(i)
                emit_o(NT - 1)

    nc.compile()
    return nc


def _get_program(flags):
    if flags not in _BUILD_CACHE:
        _BUILD_CACHE[flags] = _build_program(flags)
    return _BUILD_CACHE[flags]


def _phm_weight(A, S):
    f = A.shape[0]
    din, dout = f * S.shape[1], f * S.shape[2]
    W = np.einsum('nij,nkl->ikjl', np.asarray(A, np.float32), np.asarray(S, np.float32))
    return np.ascontiguousarray(W.reshape(din, dout))


_IDENT = np.eye(128, dtype=BF16)


def kernel(**inputs):
    query = np.asarray(inputs['query'], np.float32)
    g1 = np.asarray(inputs['g1'], np.float32)
    b1 = np.asarray(inputs['b1'], np.float32)
    g2 = np.asarray(inputs['g2'], np.float32)
    b2 = np.asarray(inputs['b2'], np.float32)
    qb = np.asarray(inputs['qb'], np.float32)
    kb = np.asarray(inputs['kb'], np.float32)
    vb = np.asarray(inputs['vb'], np.float32)
    ob = np.asarray(inputs['ob'], np.float32)

    Wq = _phm_weight(inputs['qA'], inputs['qS'])
    Wk = _phm_weight(inputs['kA'], inputs['kS'])
    Wv = _phm_weight(inputs['vA'], inputs['vS'])
    Wo = _phm_weight(inputs['oA'], inputs['oS'])
    WoI = Wo + np.eye(E, dtype=np.float32)
    Wo2 = g2[:, None] * WoI
    B2O = b2 @ WoI + ob

    # fold LN1 affine into the q projection: (qn*g1+b1) @ Wq
    Wq_eff = g1[:, None] * Wq
    qb_eff = qb + b1 @ Wq
    cq = Wq_eff.sum(axis=0)

    has_g1b1 = not (np.all(g1 == 1.0) and np.all(b1 == 0.0))
    has_qb = bool(np.any(qb_eff != 0.0))
    has_kb = bool(np.any(kb != 0.0))
    has_vb = bool(np.any(vb != 0.0))
    has_b2o = bool(np.any(B2O != 0.0))
    flags = (has_g1b1, has_qb, has_kb, has_vb, has_b2o)

    nc = _get_program(flags)

    s_full = np.sin((np.pi / 2) * np.arange(1, L + 1, dtype=np.float32) / L)
    c_full = np.cos((np.pi / 2) * np.arange(1, L + 1, dtype=np.float32) / L)

    wq_b = Wq_eff.astype(BF16)
    wk_b = Wk.astype(BF16)
    wv_b = Wv.astype(BF16)
    wo_b = Wo2.astype(BF16)

    in_maps = []
    for core in range(NCORES):
        b = core // 2
        l0 = (core % 2) * T
        x = np.ascontiguousarray(query[l0:l0 + T, b, :])
        s = s_full[l0:l0 + T]
        c = c_full[l0:l0 + T]
        im = {
            'x_tmb': x.astype(BF16),
            'x_fm': np.ascontiguousarray(x.T).astype(BF16),
            'wq': wq_b, 'wk': wk_b, 'wv': wv_b, 'wo2': wo_b,
            's_bcast': np.ascontiguousarray(np.broadcast_to(s, (128, T))).astype(BF16),
            'c_bcast': np.ascontiguousarray(np.broadcast_to(c, (128, T))).astype(BF16),
            's_cols': np.ascontiguousarray(s.reshape(NT, 128).T),
            'c_cols': np.ascontiguousarray(c.reshape(NT, 128).T),
            'cq_row': np.ascontiguousarray(cq.reshape(1, E)).astype(BF16),
            'ident': _IDENT,
        }
        if has_g1b1:
            im['g1_b'] = np.ascontiguousarray(np.broadcast_to(g1, (128, E)))
            im['b1_b'] = np.ascontiguousarray(np.broadcast_to(b1, (128, E)))
        if has_qb:
            im['qb_cols'] = np.ascontiguousarray(qb_eff.reshape(NJ, 128).T)
        if has_kb:
            im['kb_b'] = np.ascontiguousarray(np.broadcast_to(kb, (128, E)))
        if has_vb:
            im['vb_b'] = np.ascontiguousarray(np.broadcast_to(vb, (128, E)))
        if has_b2o:
            im['b2o_b'] = np.ascontiguousarray(np.broadcast_to(B2O, (128, E)))
        in_maps.append(im)

    trace = bool(os.environ.get('KERNEL_TRACE'))
    res = run_bass_kernel_spmd(nc, in_maps, list(range(NCORES)), trace=trace)
    kernel._last_exec_ns = res.exec_time_ns

    out = np.empty((L, N, E), np.float32)
    for core in range(NCORES):
        b = core // 2
        l0 = (core % 2) * T
        out[l0:l0 + T, b, :] = res.results[core]['out']
    return out


kernel._last_exec_ns = None
